# revision 2
# baseline (speedup 1.0000x reference)
"""Trainium2 Bass kernel for nn_Block_21406117003497 (dense transformer block).

B=4, T=2048, C=1024, H=16 heads, HS=64, DFF=4096.
8 cores: core c -> batch c//2, token-half c%2 (causally balanced row split).
Each core computes LN1+K/V for the full sequence of its batch (duplicated
within the pair -> zero inter-core communication), Q/attention/proj/MLP for
its own 1024 rows. Matmuls in bf16, accumulation in f32.
"""

import functools
from contextlib import ExitStack

import numpy as np
import ml_dtypes

import concourse.bass as bass
import concourse.mybir as mybir
import concourse.tile as tile
from concourse import bacc
from concourse.bass_utils import run_bass_kernel_spmd

F32 = mybir.dt.float32
BF16 = mybir.dt.bfloat16
AF = mybir.ActivationFunctionType
ALU = mybir.AluOpType
AX = mybir.AxisListType

B, T, C, H, HS = 4, 2048, 1024, 16, 64
DFF = 4 * C
R = 1024            # own rows per core
EPS = 1e-5
SCALE = float(C) ** -0.5
BF = ml_dtypes.bfloat16


def own_ranges(sub):
    """local row block -> absolute row ranges per sub (causally balanced)."""
    if sub == 0:
        return (0, 512), (1536, 2048)
    return (512, 1024), (1024, 1536)


def build_program(apply_ln_affine: bool, add_bproj: bool, add_b2: bool, repeat: int = 1):
    nc = bacc.Bacc(None, target_bir_lowering=False, debug=False)

    env = {}
    env["apply_ln_affine"] = apply_ln_affine
    env["x_kv"] = nc.dram_tensor("x_kv", [T, C], F32, kind="ExternalInput")
    env["x_own"] = nc.dram_tensor("x_own", [R, C], F32, kind="ExternalInput")
    env["maskt"] = nc.dram_tensor("maskt", [T, 512], BF16, kind="ExternalInput")
    env["wq_d"] = nc.dram_tensor("wq", [C, C], BF16, kind="ExternalInput")
    env["wk_d"] = nc.dram_tensor("wk", [C, C], BF16, kind="ExternalInput")
    env["wv_d"] = nc.dram_tensor("wv", [C, C], BF16, kind="ExternalInput")
    env["wp_d"] = nc.dram_tensor("wp", [C, C], BF16, kind="ExternalInput")
    env["w1_d"] = nc.dram_tensor("w1", [C, DFF], BF16, kind="ExternalInput")
    env["w2_d"] = nc.dram_tensor("w2", [DFF, C], BF16, kind="ExternalInput")
    env["b1r_d"] = nc.dram_tensor("b1r", [128, DFF // 128], F32, kind="ExternalInput")
    env["ln_d"] = nc.dram_tensor("lnp", [4, C], F32, kind="ExternalInput")
    env["bpb2_d"] = nc.dram_tensor("bpb2", [2, C], F32, kind="ExternalInput")
    env["out_d"] = nc.dram_tensor("out", [R, C], F32, kind="ExternalOutput")

    with tile.TileContext(nc) as tc:
        with tc.tile_pool(name="consts", bufs=1, side="left") as consts:
            env["eps_t"] = consts.tile([128, 1], F32, name="eps_t")
            nc.vector.memset(env["eps_t"][:], EPS)
            env["zeros_t"] = consts.tile([128, 512], F32, name="zeros_t")
            nc.vector.memset(env["zeros_t"][:], 0.0)
            env["b1r_t"] = consts.tile([128, DFF // 128], F32, name="b1r_t")
            nc.sync.dma_start(out=env["b1r_t"][:], in_=env["b1r_d"][:, :])
            lnb = {}
            if apply_ln_affine:
                for i, nm in enumerate(("g1", "b1", "g2", "b2")):
                    row = consts.tile([1, C], F32, name=f"lnrow_{nm}")
                    nc.sync.dma_start(out=row[:], in_=env["ln_d"][i:i + 1, :])
                    bc = consts.tile([128, C], F32, name=f"lnb_{nm}")
                    nc.gpsimd.partition_broadcast(bc[:], row[0:1, :])
                    lnb[nm] = bc
            env["lnb"] = lnb
            bias_b = {}
            for i, (nm, need) in enumerate((("bproj", add_bproj), ("b2", add_b2))):
                if need:
                    row = consts.tile([1, C], F32, name=f"brow_{nm}")
                    nc.sync.dma_start(out=row[:], in_=env["bpb2_d"][i:i + 1, :])
                    bc = consts.tile([128, C], F32, name=f"bias_{nm}")
                    nc.gpsimd.partition_broadcast(bc[:], row[0:1, :])
                    bias_b[nm] = bc
            env["bias_b"] = bias_b

            for _rep in range(repeat):
                emit_block(nc, tc, env)
    nc.compile()
    return nc


def emit_block(nc, tc, env):
    x_kv, x_own, maskt = env["x_kv"], env["x_own"], env["maskt"]
    wq_d, wk_d, wv_d, wp_d = env["wq_d"], env["wk_d"], env["wv_d"], env["wp_d"]
    w1_d, w2_d, b1r_t = env["w1_d"], env["w2_d"], env["b1r_t"]
    out_d = env["out_d"]
    eps_t, zeros_t = env["eps_t"], env["zeros_t"]
    lnb, bias_b = env["lnb"], env["bias_b"]
    apply_ln_affine = env["apply_ln_affine"]

    def layernorm_rows(src_getter, n_tiles, dst_tiles, pools, gkey, bkey):
        """LN over row-major f32 [128,C] tiles -> transposed bf16 dst tiles."""
        xc_p, st_p, rm_p = pools
        for i in range(n_tiles):
            xt = src_getter(i)
            musum = st_p.tile([128, 1], F32, name="musum", tag="musum")
            nc.vector.reduce_sum(musum[:], xt[:], axis=AX.X)
            nmu = st_p.tile([128, 1], F32, name="nmu", tag="nmu")
            nc.vector.tensor_scalar(out=nmu[:], in0=musum[:], scalar1=-1.0 / C,
                                    scalar2=None, op0=ALU.mult)
            xc = xc_p.tile([128, C], F32, name="xc", tag="xc")
            nc.scalar.activation(xc[:], xt[:], AF.Identity, bias=nmu[:, 0:1])
            sqv = xc_p.tile([128, C], F32, name="sqv", tag="sqv")
            ssq = st_p.tile([128, 1], F32, name="ssq", tag="ssq")
            nc.scalar.activation(sqv[:], xc[:], AF.Square, bias=0.0, accum_out=ssq[:])
            std = st_p.tile([128, 1], F32, name="std", tag="std")
            nc.scalar.activation(std[:], ssq[:], AF.Sqrt, bias=eps_t[:, 0:1],
                                 scale=1.0 / C)
            rstd = st_p.tile([128, 1], F32, name="rstd", tag="rstd")
            nc.vector.reciprocal(rstd[:], std[:])
            hrow = rm_p.tile([128, C], BF16, name="hrow", tag="hrow")
            if apply_ln_affine:
                tmp = xc_p.tile([128, C], F32, name="lntmp", tag="lntmp")
                nc.vector.scalar_tensor_tensor(tmp[:], xc[:], rstd[:, 0:1],
                                               lnb[gkey][:], op0=ALU.mult, op1=ALU.mult)
                nc.vector.scalar_tensor_tensor(hrow[:], tmp[:], 1.0,
                                               lnb[bkey][:], op0=ALU.mult, op1=ALU.add)
            else:
                nc.vector.tensor_scalar(out=hrow[:], in0=xc[:], scalar1=rstd[:, 0:1],
                                        scalar2=None, op0=ALU.mult)
            for cj in range(8):
                nc.sync.dma_start_transpose(
                    dst_tiles[cj][:, i * 128:(i + 1) * 128],
                    hrow[:, cj * 128:(cj + 1) * 128])

    def load_w(pool, dram, tag):
        tiles = []
        for k in range(8):
            wt = pool.tile([128, C], BF16, name=f"{tag}{k}", tag="wsb")
            nc.sync.dma_start(out=wt[:], in_=dram[k * 128:(k + 1) * 128, :])
            tiles.append(wt)
        return tiles

    # ================= LEFT: hlnT/hoT =================
    with ExitStack() as es_hln:
        p_hlnT = es_hln.enter_context(tc.tile_pool(name="p_hlnT", bufs=8, side="left"))
        p_hoT = es_hln.enter_context(tc.tile_pool(name="p_hoT", bufs=8, side="left"))
        hlnT = [p_hlnT.tile([128, T], BF16, name=f"hlnT{i}", tag="hlnT") for i in range(8)]
        hoT = [p_hoT.tile([128, R], BF16, name=f"hoT{i}", tag="hoT") for i in range(8)]

        # ---- LN1 (x_kv -> hlnT ; x_own -> hoT), locals on right ----
        with tc.tile_pool(name="p_xin", bufs=3, side="right") as p_xin, \
             tc.tile_pool(name="p_xc", bufs=3, side="right") as p_xc, \
             tc.tile_pool(name="p_st", bufs=4, side="right") as p_st, \
             tc.tile_pool(name="p_rm", bufs=3, side="right") as p_rm:
            def load_kv(i):
                xt = p_xin.tile([128, C], F32, name="xkv_t", tag="xin")
                nc.sync.dma_start(out=xt[:], in_=x_kv[i * 128:(i + 1) * 128, :])
                return xt
            layernorm_rows(load_kv, T // 128, hlnT, (p_xc, p_st, p_rm), "g1", "b1")

            def load_own(i):
                xt = p_xin.tile([128, C], F32, name="xown_t", tag="xin")
                nc.sync.dma_start(out=xt[:], in_=x_own[i * 128:(i + 1) * 128, :])
                return xt
            layernorm_rows(load_own, R // 128, hoT, (p_xc, p_st, p_rm), "g1", "b1")

        # ================= RIGHT: QKV outputs =================
        es_qkv = ExitStack()
        p_QT = es_qkv.enter_context(tc.tile_pool(name="p_QT", bufs=8, side="right"))
        p_KT = es_qkv.enter_context(tc.tile_pool(name="p_KT", bufs=8, side="right"))
        p_V = es_qkv.enter_context(tc.tile_pool(name="p_V", bufs=16, side="right"))
        QT = [p_QT.tile([128, R], BF16, name=f"QT{i}", tag="QT") for i in range(8)]
        KT = [p_KT.tile([128, T], BF16, name=f"KT{i}", tag="KT") for i in range(8)]
        Vg = [p_V.tile([128, 16 * 65], BF16, name=f"Vg{i}", tag="Vg") for i in range(16)]

        # ---- QKV matmuls (locals nested on right) ----
        with tc.tile_pool(name="p_wqkv", bufs=9, side="right") as p_w, \
             tc.tile_pool(name="ps_qkv", bufs=4, space="PSUM") as ps_a:
            for dram, dstT, rhsT, n_tok in ((wq_d, QT, hoT, R), (wk_d, KT, hlnT, T)):
                wt = load_w(p_w, dram, "wqk")
                for m in range(8):
                    for nbp in range(n_tok // 1024):
                        psA = ps_a.tile([128, 512], F32, name="qk_psA", tag="ps_a")
                        psB = ps_a.tile([128, 512], F32, name="qk_psB", tag="ps_a")
                        for k in range(8):
                            lhs = wt[k][:, m * 128:(m + 1) * 128]
                            nc.tensor.matmul(psA[:], lhs,
                                             rhsT[k][:, (2 * nbp) * 512:(2 * nbp + 1) * 512],
                                             start=(k == 0), stop=(k == 7))
                            nc.tensor.matmul(psB[:], lhs,
                                             rhsT[k][:, (2 * nbp + 1) * 512:(2 * nbp + 2) * 512],
                                             start=(k == 0), stop=(k == 7))
                        nc.vector.tensor_copy(dstT[m][:, (2 * nbp) * 512:(2 * nbp + 1) * 512], psA[:])
                        nc.vector.tensor_copy(dstT[m][:, (2 * nbp + 1) * 512:(2 * nbp + 2) * 512], psB[:])

            wt = load_w(p_w, wv_d, "wv")
            for tch in range(16):
                nc.gpsimd.memset(
                    Vg[tch][:, 0:16 * 65].rearrange("p (h d) -> p h d", d=65)[:, :, 64:65], 1.0)
                psA = ps_a.tile([128, 512], F32, name="v_psA", tag="ps_a")
                psB = ps_a.tile([128, 512], F32, name="v_psB", tag="ps_a")
                for k in range(8):
                    lhs = hlnT[k][:, tch * 128:(tch + 1) * 128]
                    nc.tensor.matmul(psA[:], lhs, wt[k][:, 0:512], start=(k == 0), stop=(k == 7))
                    nc.tensor.matmul(psB[:], lhs, wt[k][:, 512:1024], start=(k == 0), stop=(k == 7))
                for j, ps in ((0, psA), (1, psB)):
                    dst = Vg[tch][:, j * 8 * 65:(j + 1) * 8 * 65].rearrange(
                        "p (h d) -> p h d", d=65)[:, :, 0:64]
                    nc.vector.tensor_copy(dst, ps[:].rearrange("p (h d) -> p h d", d=64))
    # es_hln closed here (hlnT/hoT freed, left side)

    # ================= LEFT: oT =================
    with ExitStack() as es_oT:
        p_oT = es_oT.enter_context(tc.tile_pool(name="p_oT", bufs=8, side="left"))
        oT = [p_oT.tile([128, R], BF16, name=f"oT{i}", tag="oT") for i in range(8)]

        # ---- attention (locals nested on right, inside QKV-outs) ----
        with tc.tile_pool(name="p_mask", bufs=16, side="right") as p_mask, \
             tc.tile_pool(name="p_E", bufs=6, side="right") as p_E, \
             tc.tile_pool(name="p_inv", bufs=4, side="right") as p_inv, \
             tc.tile_pool(name="ps_s", bufs=4, space="PSUM") as ps_s, \
             tc.tile_pool(name="ps_av", bufs=4, space="PSUM") as ps_av:
            mk = []
            for kt in range(16):
                m = p_mask.tile([128, 512], BF16, name=f"mk{kt}", tag="mk")
                nc.sync.dma_start(out=m[:], in_=maskt[kt * 128:(kt + 1) * 128, :])
                mk.append(m)

            for hp in range(8):
                for b in (0, 1):
                    n_kt = 8 if b == 0 else 16
                    oa = ps_av.tile([128, 512], F32, name="av_psA", tag="ps_av")
                    ob = ps_av.tile([128, 512], F32, name="av_psB", tag="ps_av")
                    for kt in range(n_kt):
                        masked = (b == 0) or (kt >= 8)
                        Es = []
                        for hh in (0, 1):
                            kslc = KT[hp][hh * 64:(hh + 1) * 64, kt * 128:(kt + 1) * 128]
                            qslc = QT[hp][hh * 64:(hh + 1) * 64, b * 512:(b + 1) * 512]
                            sps = ps_s.tile([128, 512], F32, name="s_ps", tag="ps_s")
                            nc.tensor.matmul(sps[:], kslc, qslc, start=True, stop=True)
                            E = p_E.tile([128, 512], BF16, name="E", tag="E")
                            nc.scalar.activation(E[:], sps[:], AF.Exp, scale=SCALE)
                            if masked:
                                nc.gpsimd.tensor_mul(E[:], E[:], mk[kt][:])
                            Es.append(E)
                        for hh, ops in ((0, oa), (1, ob)):
                            h = 2 * hp + hh
                            nc.tensor.matmul(ops[0:65, :], Vg[kt][:, h * 65:h * 65 + 65],
                                             Es[hh][:], start=(kt == 0), stop=(kt == n_kt - 1))
                    for hh, ops in ((0, oa), (1, ob)):
                        invd = p_inv.tile([1, 512], F32, name="invd", tag="invd")
                        nc.vector.reciprocal(invd[:], ops[64:65, :])
                        invb = p_inv.tile([64, 512], F32, name="invb", tag="invb")
                        nc.gpsimd.partition_broadcast(invb[:], invd[0:1, :])
                        nc.vector.tensor_mul(
                            oT[hp][hh * 64:(hh + 1) * 64, b * 512:(b + 1) * 512],
                            ops[0:64, :], invb[:])

        es_qkv.close()  # QT/KT/V freed (right side)

        # ================= RIGHT: x2 =================
        es_x2 = ExitStack()
        p_x2 = es_x2.enter_context(tc.tile_pool(name="p_x2", bufs=8, side="right"))
        x2 = [p_x2.tile([128, C], F32, name=f"x2_{i}", tag="x2") for i in range(8)]

        # ---- proj + residual (locals nested on right, inside x2) ----
        with tc.tile_pool(name="p_wp", bufs=9, side="right") as p_wp, \
             tc.tile_pool(name="p_xo", bufs=3, side="right") as p_xo, \
             tc.tile_pool(name="ps_pj", bufs=4, space="PSUM") as ps_pj:
            wt = load_w(p_wp, wp_d, "wp")
            for tch in range(8):
                xo = p_xo.tile([128, C], F32, name="xo", tag="xo")
                nc.sync.dma_start(out=xo[:], in_=x_own[tch * 128:(tch + 1) * 128, :])
                psA = ps_pj.tile([128, 512], F32, name="pj_psA", tag="ps_pj")
                psB = ps_pj.tile([128, 512], F32, name="pj_psB", tag="ps_pj")
                for k in range(8):
                    lhs = oT[k][:, tch * 128:(tch + 1) * 128]
                    nc.tensor.matmul(psA[:], lhs, wt[k][:, 0:512], start=(k == 0), stop=(k == 7))
                    nc.tensor.matmul(psB[:], lhs, wt[k][:, 512:1024], start=(k == 0), stop=(k == 7))
                for j, ps in ((0, psA), (1, psB)):
                    sl = slice(j * 512, (j + 1) * 512)
                    nc.vector.scalar_tensor_tensor(x2[tch][:, sl], ps[:], 1.0, xo[:, sl],
                                                   op0=ALU.mult, op1=ALU.add)
                    if "bproj" in bias_b:
                        nc.vector.scalar_tensor_tensor(x2[tch][:, sl], x2[tch][:, sl], 1.0,
                                                       bias_b["bproj"][:, sl],
                                                       op0=ALU.mult, op1=ALU.add)
    # es_oT closed (left)

    # ================= LEFT: h2T =================
    with ExitStack() as es_h2:
        p_h2T = es_h2.enter_context(tc.tile_pool(name="p_h2T", bufs=8, side="left"))
        h2T = [p_h2T.tile([128, R], BF16, name=f"h2T{i}", tag="h2T") for i in range(8)]
        with tc.tile_pool(name="p_xc2", bufs=3, side="left") as p_xc2, \
             tc.tile_pool(name="p_st2", bufs=4, side="left") as p_st2, \
             tc.tile_pool(name="p_rm2", bufs=3, side="left") as p_rm2:
            layernorm_rows(lambda i: x2[i], 8, h2T, (p_xc2, p_st2, p_rm2), "g2", "b2")

        # ================= RIGHT: relu1T (inside x2) =================
        es_r1 = ExitStack()
        p_r1 = es_r1.enter_context(tc.tile_pool(name="p_r1", bufs=32, side="right"))
        r1T = [p_r1.tile([128, R], BF16, name=f"r1T{i}", tag="r1T") for i in range(32)]
        ps_m = es_r1.enter_context(tc.tile_pool(name="ps_m", bufs=4, space="PSUM"))

        with tc.tile_pool(name="p_w1", bufs=16, side="right") as p_w1:
            for dblock in range(8):
                w1c = []
                for k in range(8):
                    wt1 = p_w1.tile([128, 512], BF16, name=f"w1c{dblock}_{k}", tag="w1c")
                    nc.sync.dma_start(out=wt1[:], in_=w1_d[k * 128:(k + 1) * 128,
                                                           dblock * 512:(dblock + 1) * 512])
                    w1c.append(wt1)
                for dc in range(4):
                    g = dblock * 4 + dc
                    psA = ps_m.tile([128, 512], F32, name="m1_psA", tag="ps_m")
                    psB = ps_m.tile([128, 512], F32, name="m1_psB", tag="ps_m")
                    for k in range(8):
                        lhs = w1c[k][:, dc * 128:(dc + 1) * 128]
                        nc.tensor.matmul(psA[:], lhs, h2T[k][:, 0:512],
                                         start=(k == 0), stop=(k == 7))
                        nc.tensor.matmul(psB[:], lhs, h2T[k][:, 512:1024],
                                         start=(k == 0), stop=(k == 7))
                    for j, ps in ((0, psA), (1, psB)):
                        nc.vector.scalar_tensor_tensor(
                            r1T[g][:, j * 512:(j + 1) * 512], ps[:], b1r_t[:, g:g + 1],
                            zeros_t[:], op0=ALU.add, op1=ALU.max)
    # es_h2 closed (left)

    with tc.tile_pool(name="p_w2", bufs=18, side="right") as p_w2:
        for kh in range(2):
            w2c = []
            for k in range(16):
                kk = kh * 16 + k
                wt2 = p_w2.tile([128, C], BF16, name=f"w2c{kh}_{k}", tag="w2c")
                nc.sync.dma_start(out=wt2[:], in_=w2_d[kk * 128:(kk + 1) * 128, :])
                w2c.append(wt2)
            for tch in range(8):
                psA = ps_m.tile([128, 512], F32, name="m2_psA", tag="ps_m")
                psB = ps_m.tile([128, 512], F32, name="m2_psB", tag="ps_m")
                for k in range(16):
                    kk = kh * 16 + k
                    lhs = r1T[kk][:, tch * 128:(tch + 1) * 128]
                    nc.tensor.matmul(psA[:], lhs, w2c[k][:, 0:512],
                                     start=(k == 0), stop=(k == 15))
                    nc.tensor.matmul(psB[:], lhs, w2c[k][:, 512:1024],
                                     start=(k == 0), stop=(k == 15))
                for j, ps in ((0, psA), (1, psB)):
                    sl = slice(j * 512, (j + 1) * 512)
                    nc.vector.scalar_tensor_tensor(x2[tch][:, sl], ps[:], 1.0,
                                                   x2[tch][:, sl], op0=ALU.mult, op1=ALU.add)
                if kh == 1:
                    if "b2" in bias_b:
                        for j in range(2):
                            sl = slice(j * 512, (j + 1) * 512)
                            nc.vector.scalar_tensor_tensor(
                                x2[tch][:, sl], x2[tch][:, sl], 1.0,
                                bias_b["b2"][:, sl], op0=ALU.mult, op1=ALU.add)
                    nc.sync.dma_start(out=out_d[tch * 128:(tch + 1) * 128, :], in_=x2[tch][:])

    es_r1.close()
    es_x2.close()


@functools.lru_cache(maxsize=4)
def _cached_program(apply_ln_affine, add_bproj, add_b2, repeat):
    return build_program(apply_ln_affine, add_bproj, add_b2, repeat)


def _prep_shards(x, Wq, Wk, Wv, Wproj, bproj, ln1_g, ln1_b, ln2_g, ln2_b, W1, b1, W2, b2):
    wq = np.ascontiguousarray(Wq.transpose(1, 0, 2).reshape(C, C)).astype(BF)
    wk = np.ascontiguousarray(Wk.transpose(1, 0, 2).reshape(C, C)).astype(BF)
    wv = np.ascontiguousarray(Wv.transpose(1, 0, 2).reshape(C, C)).astype(BF)
    wp = Wproj.astype(BF)
    w1 = W1.astype(BF)
    w2 = W2.astype(BF)
    b1r = np.ascontiguousarray(b1.reshape(DFF // 128, 128).T).astype(np.float32)
    lnp = np.stack([ln1_g, ln1_b, ln2_g, ln2_b]).astype(np.float32)
    bpb2 = np.stack([bproj, b2]).astype(np.float32)

    in_maps = []
    for c in range(8):
        bidx, sub = c // 2, c % 2
        (lo0, lo1), (hi0, hi1) = own_ranges(sub)
        xb = x[bidx]
        x_own = np.concatenate([xb[lo0:lo1], xb[hi0:hi1]], axis=0).astype(np.float32)
        keys = np.arange(T)
        rows_b0 = np.arange(lo0, lo1)
        rows_b1 = np.arange(hi0, hi1)
        m = np.zeros((T, 512), np.float32)
        m[0:1024] = (keys[0:1024, None] <= rows_b0[None, :])
        m[1024:2048] = (keys[1024:2048, None] <= rows_b1[None, :])
        in_maps.append({
            "x_kv": np.ascontiguousarray(xb).astype(np.float32),
            "x_own": x_own,
            "maskt": m.astype(BF),
            "wq": wq, "wk": wk, "wv": wv, "wp": wp,
            "w1": w1, "w2": w2, "b1r": b1r, "lnp": lnp, "bpb2": bpb2,
        })
    return in_maps


def kernel(repeat: int = 1, **inputs) -> np.ndarray:
    inputs = {k: np.asarray(v) for k, v in inputs.items()}
    apply_ln_affine = not (
        np.all(inputs["ln1_g"] == 1) and np.all(inputs["ln1_b"] == 0)
        and np.all(inputs["ln2_g"] == 1) and np.all(inputs["ln2_b"] == 0))
    add_bproj = bool(np.any(inputs["bproj"] != 0))
    add_b2 = bool(np.any(inputs["b2"] != 0))
    nc = _cached_program(apply_ln_affine, add_bproj, add_b2, repeat)
    in_maps = _prep_shards(**inputs)
    res = run_bass_kernel_spmd(nc, in_maps, list(range(8)))
    out = np.empty((B, T, C), np.float32)
    for c in range(8):
        bidx, sub = c // 2, c % 2
        (lo0, lo1), (hi0, hi1) = own_ranges(sub)
        oc = res.results[c]["out"]
        out[bidx, lo0:lo1] = oc[0:512]
        out[bidx, hi0:hi1] = oc[512:1024]
    return out


# revision 6
# speedup vs baseline: 425.2109x; 425.2109x over previous
"""Trainium2 Bass kernel for nn_Block_21406117003497 (dense transformer block).

B=4, T=2048, C=1024, H=16 heads, HS=64, DFF=4096.
8 cores: core c -> batch c//2, token-half c%2 (causally balanced row split).
Each core computes LN1+K/V for the full sequence of its batch (duplicated
within the pair -> zero inter-core communication), Q/attention/proj/MLP for
its own 1024 rows. Matmuls in bf16, accumulation in f32.
"""

import functools
from contextlib import ExitStack

import numpy as np
import ml_dtypes

import concourse.bass as bass
import concourse.mybir as mybir
import concourse.tile as tile
from concourse import bacc
from concourse.bass_utils import run_bass_kernel_spmd

F32 = mybir.dt.float32
BF16 = mybir.dt.bfloat16
AF = mybir.ActivationFunctionType
ALU = mybir.AluOpType
AX = mybir.AxisListType

B, T, C, H, HS = 4, 2048, 1024, 16, 64
DFF = 4 * C
R = 1024            # own rows per core
EPS = 1e-5
SCALE = float(C) ** -0.5
BF = ml_dtypes.bfloat16


def own_ranges(sub):
    """local row block -> absolute row ranges per sub (causally balanced)."""
    if sub == 0:
        return (0, 512), (1536, 2048)
    return (512, 1024), (1024, 1536)


def build_program(apply_ln_affine: bool, add_bproj: bool, add_b2: bool, repeat: int = 1,
                  loop_n: int = 0):
    nc = bacc.Bacc(None, target_bir_lowering=False, debug=False)

    env = {}
    env["apply_ln_affine"] = apply_ln_affine
    env["x_kv"] = nc.dram_tensor("x_kv", [T, C], F32, kind="ExternalInput")
    env["x_own"] = nc.dram_tensor("x_own", [R, C], F32, kind="ExternalInput")
    env["maskt"] = nc.dram_tensor("maskt", [T, 512], BF16, kind="ExternalInput")
    env["wq_d"] = nc.dram_tensor("wq", [C, C], BF16, kind="ExternalInput")
    env["wk_d"] = nc.dram_tensor("wk", [C, C], BF16, kind="ExternalInput")
    env["wv_d"] = nc.dram_tensor("wv", [C, C], BF16, kind="ExternalInput")
    env["wp_d"] = nc.dram_tensor("wp", [C, C], BF16, kind="ExternalInput")
    env["w1_d"] = nc.dram_tensor("w1", [C, DFF], BF16, kind="ExternalInput")
    env["w2_d"] = nc.dram_tensor("w2", [DFF, C], BF16, kind="ExternalInput")
    env["b1r_d"] = nc.dram_tensor("b1r", [128, DFF // 128], F32, kind="ExternalInput")
    env["ln_d"] = nc.dram_tensor("lnp", [4, C], F32, kind="ExternalInput")
    env["bpb2_d"] = nc.dram_tensor("bpb2", [2, C], F32, kind="ExternalInput")
    env["out_d"] = nc.dram_tensor("out", [R, C], F32, kind="ExternalOutput")

    with tile.TileContext(nc) as tc:
        with tc.tile_pool(name="consts", bufs=1, side="left") as consts:
            env["eps_t"] = consts.tile([128, 1], F32, name="eps_t")
            nc.vector.memset(env["eps_t"][:], EPS)
            env["zeros_t"] = consts.tile([128, 512], F32, name="zeros_t")
            nc.vector.memset(env["zeros_t"][:], 0.0)
            env["b1r_t"] = consts.tile([128, DFF // 128], F32, name="b1r_t")
            nc.sync.dma_start(out=env["b1r_t"][:], in_=env["b1r_d"][:, :])
            lnb = {}
            if apply_ln_affine:
                for i, nm in enumerate(("g1", "b1", "g2", "b2")):
                    row = consts.tile([1, C], F32, name=f"lnrow_{nm}")
                    nc.sync.dma_start(out=row[:], in_=env["ln_d"][i:i + 1, :])
                    bc = consts.tile([128, C], F32, name=f"lnb_{nm}")
                    nc.gpsimd.partition_broadcast(bc[:], row[0:1, :])
                    lnb[nm] = bc
            env["lnb"] = lnb
            bias_b = {}
            for i, (nm, need) in enumerate((("bproj", add_bproj), ("b2", add_b2))):
                if need:
                    row = consts.tile([1, C], F32, name=f"brow_{nm}")
                    nc.sync.dma_start(out=row[:], in_=env["bpb2_d"][i:i + 1, :])
                    bc = consts.tile([128, C], F32, name=f"bias_{nm}")
                    nc.gpsimd.partition_broadcast(bc[:], row[0:1, :])
                    bias_b[nm] = bc
            env["bias_b"] = bias_b

            if loop_n:
                with tc.For_i(0, loop_n, 1):
                    emit_block(nc, tc, env)
            else:
                for _rep in range(repeat):
                    emit_block(nc, tc, env)
    nc.compile()
    return nc


def emit_block(nc, tc, env):
    x_kv, x_own, maskt = env["x_kv"], env["x_own"], env["maskt"]
    wq_d, wk_d, wv_d, wp_d = env["wq_d"], env["wk_d"], env["wv_d"], env["wp_d"]
    w1_d, w2_d, b1r_t = env["w1_d"], env["w2_d"], env["b1r_t"]
    out_d = env["out_d"]
    eps_t, zeros_t = env["eps_t"], env["zeros_t"]
    lnb, bias_b = env["lnb"], env["bias_b"]
    apply_ln_affine = env["apply_ln_affine"]

    def layernorm_rows(src_getter, n_tiles, dst_tiles, pools, gkey, bkey):
        """LN over row-major f32 [128,C] tiles -> transposed bf16 dst tiles."""
        xc_p, st_p, rm_p = pools
        for i in range(n_tiles):
            xt = src_getter(i)
            musum = st_p.tile([128, 1], F32, name="musum", tag="musum")
            nc.vector.reduce_sum(musum[:], xt[:], axis=AX.X)
            nmu = st_p.tile([128, 1], F32, name="nmu", tag="nmu")
            nc.vector.tensor_scalar(out=nmu[:], in0=musum[:], scalar1=-1.0 / C,
                                    scalar2=None, op0=ALU.mult)
            xc = xc_p.tile([128, C], F32, name="xc", tag="xc")
            nc.scalar.activation(xc[:], xt[:], AF.Identity, bias=nmu[:, 0:1])
            sqv = xc_p.tile([128, C], F32, name="sqv", tag="sqv")
            ssq = st_p.tile([128, 1], F32, name="ssq", tag="ssq")
            nc.scalar.activation(sqv[:], xc[:], AF.Square, bias=0.0, accum_out=ssq[:])
            std = st_p.tile([128, 1], F32, name="std", tag="std")
            nc.scalar.activation(std[:], ssq[:], AF.Sqrt, bias=eps_t[:, 0:1],
                                 scale=1.0 / C)
            rstd = st_p.tile([128, 1], F32, name="rstd", tag="rstd")
            nc.vector.reciprocal(rstd[:], std[:])
            hrow = rm_p.tile([128, C], BF16, name="hrow", tag="hrow")
            if apply_ln_affine:
                tmp = xc_p.tile([128, C], F32, name="lntmp", tag="lntmp")
                nc.vector.scalar_tensor_tensor(tmp[:], xc[:], rstd[:, 0:1],
                                               lnb[gkey][:], op0=ALU.mult, op1=ALU.mult)
                nc.vector.scalar_tensor_tensor(hrow[:], tmp[:], 1.0,
                                               lnb[bkey][:], op0=ALU.mult, op1=ALU.add)
            else:
                nc.vector.tensor_scalar(out=hrow[:], in0=xc[:], scalar1=rstd[:, 0:1],
                                        scalar2=None, op0=ALU.mult)
            for cj in range(8):
                nc.sync.dma_start_transpose(
                    dst_tiles[cj][:, i * 128:(i + 1) * 128],
                    hrow[:, cj * 128:(cj + 1) * 128])

    def load_w(pool, dram, tag):
        tiles = []
        for k in range(8):
            wt = pool.tile([128, C], BF16, name=f"{tag}{k}", tag="wsb")
            nc.sync.dma_start(out=wt[:], in_=dram[k * 128:(k + 1) * 128, :])
            tiles.append(wt)
        return tiles

    # ================= LEFT: hlnT/hoT =================
    with ExitStack() as es_hln:
        p_hlnT = es_hln.enter_context(tc.tile_pool(name="p_hlnT", bufs=8, side="left"))
        p_hoT = es_hln.enter_context(tc.tile_pool(name="p_hoT", bufs=8, side="left"))
        hlnT = [p_hlnT.tile([128, T], BF16, name=f"hlnT{i}", tag="hlnT") for i in range(8)]
        hoT = [p_hoT.tile([128, R], BF16, name=f"hoT{i}", tag="hoT") for i in range(8)]

        # ---- LN1 (x_kv -> hlnT ; x_own -> hoT), locals on right ----
        with tc.tile_pool(name="p_xin", bufs=3, side="right") as p_xin, \
             tc.tile_pool(name="p_xc", bufs=3, side="right") as p_xc, \
             tc.tile_pool(name="p_st", bufs=4, side="right") as p_st, \
             tc.tile_pool(name="p_rm", bufs=3, side="right") as p_rm:
            def load_kv(i):
                xt = p_xin.tile([128, C], F32, name="xkv_t", tag="xin")
                nc.sync.dma_start(out=xt[:], in_=x_kv[i * 128:(i + 1) * 128, :])
                return xt
            layernorm_rows(load_kv, T // 128, hlnT, (p_xc, p_st, p_rm), "g1", "b1")

            def load_own(i):
                xt = p_xin.tile([128, C], F32, name="xown_t", tag="xin")
                nc.sync.dma_start(out=xt[:], in_=x_own[i * 128:(i + 1) * 128, :])
                return xt
            layernorm_rows(load_own, R // 128, hoT, (p_xc, p_st, p_rm), "g1", "b1")

        # ================= RIGHT: QKV outputs =================
        es_qkv = ExitStack()
        p_QT = es_qkv.enter_context(tc.tile_pool(name="p_QT", bufs=8, side="right"))
        p_KT = es_qkv.enter_context(tc.tile_pool(name="p_KT", bufs=8, side="right"))
        p_V = es_qkv.enter_context(tc.tile_pool(name="p_V", bufs=16, side="right"))
        QT = [p_QT.tile([128, R], BF16, name=f"QT{i}", tag="QT") for i in range(8)]
        KT = [p_KT.tile([128, T], BF16, name=f"KT{i}", tag="KT") for i in range(8)]
        Vg = [p_V.tile([128, 16 * 65], BF16, name=f"Vg{i}", tag="Vg") for i in range(16)]

        # ---- QKV matmuls (locals nested on right) ----
        with tc.tile_pool(name="p_wqkv", bufs=9, side="right") as p_w, \
             tc.tile_pool(name="ps_qkv", bufs=4, space="PSUM") as ps_a:
            for dram, dstT, rhsT, n_tok in ((wq_d, QT, hoT, R), (wk_d, KT, hlnT, T)):
                wt = load_w(p_w, dram, "wqk")
                for m in range(8):
                    for nbp in range(n_tok // 1024):
                        psA = ps_a.tile([128, 512], F32, name="qk_psA", tag="ps_a")
                        psB = ps_a.tile([128, 512], F32, name="qk_psB", tag="ps_a")
                        for k in range(8):
                            lhs = wt[k][:, m * 128:(m + 1) * 128]
                            nc.tensor.matmul(psA[:], lhs,
                                             rhsT[k][:, (2 * nbp) * 512:(2 * nbp + 1) * 512],
                                             start=(k == 0), stop=(k == 7))
                            nc.tensor.matmul(psB[:], lhs,
                                             rhsT[k][:, (2 * nbp + 1) * 512:(2 * nbp + 2) * 512],
                                             start=(k == 0), stop=(k == 7))
                        nc.vector.tensor_copy(dstT[m][:, (2 * nbp) * 512:(2 * nbp + 1) * 512], psA[:])
                        nc.vector.tensor_copy(dstT[m][:, (2 * nbp + 1) * 512:(2 * nbp + 2) * 512], psB[:])

            wt = load_w(p_w, wv_d, "wv")
            for tch in range(16):
                nc.gpsimd.memset(
                    Vg[tch][:, 0:16 * 65].rearrange("p (h d) -> p h d", d=65)[:, :, 64:65], 1.0)
                psA = ps_a.tile([128, 512], F32, name="v_psA", tag="ps_a")
                psB = ps_a.tile([128, 512], F32, name="v_psB", tag="ps_a")
                for k in range(8):
                    lhs = hlnT[k][:, tch * 128:(tch + 1) * 128]
                    nc.tensor.matmul(psA[:], lhs, wt[k][:, 0:512], start=(k == 0), stop=(k == 7))
                    nc.tensor.matmul(psB[:], lhs, wt[k][:, 512:1024], start=(k == 0), stop=(k == 7))
                for j, ps in ((0, psA), (1, psB)):
                    dst = Vg[tch][:, j * 8 * 65:(j + 1) * 8 * 65].rearrange(
                        "p (h d) -> p h d", d=65)[:, :, 0:64]
                    nc.vector.tensor_copy(dst, ps[:].rearrange("p (h d) -> p h d", d=64))
    # es_hln closed here (hlnT/hoT freed, left side)

    # ================= LEFT: oT =================
    with ExitStack() as es_oT:
        p_oT = es_oT.enter_context(tc.tile_pool(name="p_oT", bufs=8, side="left"))
        oT = [p_oT.tile([128, R], BF16, name=f"oT{i}", tag="oT") for i in range(8)]

        # ---- attention (locals nested on right, inside QKV-outs) ----
        with tc.tile_pool(name="p_mask", bufs=16, side="right") as p_mask, \
             tc.tile_pool(name="p_E", bufs=6, side="right") as p_E, \
             tc.tile_pool(name="p_inv", bufs=4, side="right") as p_inv, \
             tc.tile_pool(name="ps_s", bufs=4, space="PSUM") as ps_s, \
             tc.tile_pool(name="ps_av", bufs=4, space="PSUM") as ps_av:
            mk = []
            for kt in range(16):
                m = p_mask.tile([128, 512], BF16, name=f"mk{kt}", tag="mk")
                nc.sync.dma_start(out=m[:], in_=maskt[kt * 128:(kt + 1) * 128, :])
                mk.append(m)

            for hp in range(8):
                for b in (0, 1):
                    n_kt = 8 if b == 0 else 16
                    oa = ps_av.tile([128, 512], F32, name="av_psA", tag="ps_av")
                    ob = ps_av.tile([128, 512], F32, name="av_psB", tag="ps_av")
                    for kt in range(n_kt):
                        masked = (b == 0) or (kt >= 8)
                        Es = []
                        for hh in (0, 1):
                            kslc = KT[hp][hh * 64:(hh + 1) * 64, kt * 128:(kt + 1) * 128]
                            qslc = QT[hp][hh * 64:(hh + 1) * 64, b * 512:(b + 1) * 512]
                            sps = ps_s.tile([128, 512], F32, name="s_ps", tag="ps_s")
                            nc.tensor.matmul(sps[:], kslc, qslc, start=True, stop=True)
                            E = p_E.tile([128, 512], BF16, name="E", tag="E")
                            nc.scalar.activation(E[:], sps[:], AF.Exp, scale=SCALE)
                            if masked:
                                nc.gpsimd.tensor_mul(E[:], E[:], mk[kt][:])
                            Es.append(E)
                        for hh, ops in ((0, oa), (1, ob)):
                            h = 2 * hp + hh
                            nc.tensor.matmul(ops[0:65, :], Vg[kt][:, h * 65:h * 65 + 65],
                                             Es[hh][:], start=(kt == 0), stop=(kt == n_kt - 1))
                    for hh, ops in ((0, oa), (1, ob)):
                        invd = p_inv.tile([1, 512], F32, name="invd", tag="invd")
                        nc.vector.reciprocal(invd[:], ops[64:65, :])
                        invb = p_inv.tile([64, 512], F32, name="invb", tag="invb")
                        nc.gpsimd.partition_broadcast(invb[:], invd[0:1, :])
                        nc.vector.tensor_mul(
                            oT[hp][hh * 64:(hh + 1) * 64, b * 512:(b + 1) * 512],
                            ops[0:64, :], invb[:])

        es_qkv.close()  # QT/KT/V freed (right side)

        # ================= RIGHT: x2 =================
        es_x2 = ExitStack()
        p_x2 = es_x2.enter_context(tc.tile_pool(name="p_x2", bufs=8, side="right"))
        x2 = [p_x2.tile([128, C], F32, name=f"x2_{i}", tag="x2") for i in range(8)]

        # ---- proj + residual (locals nested on right, inside x2) ----
        with tc.tile_pool(name="p_wp", bufs=9, side="right") as p_wp, \
             tc.tile_pool(name="p_xo", bufs=3, side="right") as p_xo, \
             tc.tile_pool(name="ps_pj", bufs=4, space="PSUM") as ps_pj:
            wt = load_w(p_wp, wp_d, "wp")
            for tch in range(8):
                xo = p_xo.tile([128, C], F32, name="xo", tag="xo")
                nc.sync.dma_start(out=xo[:], in_=x_own[tch * 128:(tch + 1) * 128, :])
                psA = ps_pj.tile([128, 512], F32, name="pj_psA", tag="ps_pj")
                psB = ps_pj.tile([128, 512], F32, name="pj_psB", tag="ps_pj")
                for k in range(8):
                    lhs = oT[k][:, tch * 128:(tch + 1) * 128]
                    nc.tensor.matmul(psA[:], lhs, wt[k][:, 0:512], start=(k == 0), stop=(k == 7))
                    nc.tensor.matmul(psB[:], lhs, wt[k][:, 512:1024], start=(k == 0), stop=(k == 7))
                for j, ps in ((0, psA), (1, psB)):
                    sl = slice(j * 512, (j + 1) * 512)
                    nc.vector.scalar_tensor_tensor(x2[tch][:, sl], ps[:], 1.0, xo[:, sl],
                                                   op0=ALU.mult, op1=ALU.add)
                    if "bproj" in bias_b:
                        nc.vector.scalar_tensor_tensor(x2[tch][:, sl], x2[tch][:, sl], 1.0,
                                                       bias_b["bproj"][:, sl],
                                                       op0=ALU.mult, op1=ALU.add)
    # es_oT closed (left)

    # ================= LEFT: h2T =================
    with ExitStack() as es_h2:
        p_h2T = es_h2.enter_context(tc.tile_pool(name="p_h2T", bufs=8, side="left"))
        h2T = [p_h2T.tile([128, R], BF16, name=f"h2T{i}", tag="h2T") for i in range(8)]
        with tc.tile_pool(name="p_xc2", bufs=3, side="left") as p_xc2, \
             tc.tile_pool(name="p_st2", bufs=4, side="left") as p_st2, \
             tc.tile_pool(name="p_rm2", bufs=3, side="left") as p_rm2:
            layernorm_rows(lambda i: x2[i], 8, h2T, (p_xc2, p_st2, p_rm2), "g2", "b2")

        # ================= RIGHT: relu1T (inside x2) =================
        es_r1 = ExitStack()
        p_r1 = es_r1.enter_context(tc.tile_pool(name="p_r1", bufs=32, side="right"))
        r1T = [p_r1.tile([128, R], BF16, name=f"r1T{i}", tag="r1T") for i in range(32)]
        ps_m = es_r1.enter_context(tc.tile_pool(name="ps_m", bufs=4, space="PSUM"))

        with tc.tile_pool(name="p_w1", bufs=16, side="right") as p_w1:
            for dblock in range(8):
                w1c = []
                for k in range(8):
                    wt1 = p_w1.tile([128, 512], BF16, name=f"w1c{dblock}_{k}", tag="w1c")
                    nc.sync.dma_start(out=wt1[:], in_=w1_d[k * 128:(k + 1) * 128,
                                                           dblock * 512:(dblock + 1) * 512])
                    w1c.append(wt1)
                for dc in range(4):
                    g = dblock * 4 + dc
                    psA = ps_m.tile([128, 512], F32, name="m1_psA", tag="ps_m")
                    psB = ps_m.tile([128, 512], F32, name="m1_psB", tag="ps_m")
                    for k in range(8):
                        lhs = w1c[k][:, dc * 128:(dc + 1) * 128]
                        nc.tensor.matmul(psA[:], lhs, h2T[k][:, 0:512],
                                         start=(k == 0), stop=(k == 7))
                        nc.tensor.matmul(psB[:], lhs, h2T[k][:, 512:1024],
                                         start=(k == 0), stop=(k == 7))
                    for j, ps in ((0, psA), (1, psB)):
                        nc.vector.scalar_tensor_tensor(
                            r1T[g][:, j * 512:(j + 1) * 512], ps[:], b1r_t[:, g:g + 1],
                            zeros_t[:], op0=ALU.add, op1=ALU.max)
    # es_h2 closed (left)

    with tc.tile_pool(name="p_w2", bufs=18, side="right") as p_w2:
        for kh in range(2):
            w2c = []
            for k in range(16):
                kk = kh * 16 + k
                wt2 = p_w2.tile([128, C], BF16, name=f"w2c{kh}_{k}", tag="w2c")
                nc.sync.dma_start(out=wt2[:], in_=w2_d[kk * 128:(kk + 1) * 128, :])
                w2c.append(wt2)
            for tch in range(8):
                psA = ps_m.tile([128, 512], F32, name="m2_psA", tag="ps_m")
                psB = ps_m.tile([128, 512], F32, name="m2_psB", tag="ps_m")
                for k in range(16):
                    kk = kh * 16 + k
                    lhs = r1T[kk][:, tch * 128:(tch + 1) * 128]
                    nc.tensor.matmul(psA[:], lhs, w2c[k][:, 0:512],
                                     start=(k == 0), stop=(k == 15))
                    nc.tensor.matmul(psB[:], lhs, w2c[k][:, 512:1024],
                                     start=(k == 0), stop=(k == 15))
                for j, ps in ((0, psA), (1, psB)):
                    sl = slice(j * 512, (j + 1) * 512)
                    nc.vector.scalar_tensor_tensor(x2[tch][:, sl], ps[:], 1.0,
                                                   x2[tch][:, sl], op0=ALU.mult, op1=ALU.add)
                if kh == 1:
                    if "b2" in bias_b:
                        for j in range(2):
                            sl = slice(j * 512, (j + 1) * 512)
                            nc.vector.scalar_tensor_tensor(
                                x2[tch][:, sl], x2[tch][:, sl], 1.0,
                                bias_b["b2"][:, sl], op0=ALU.mult, op1=ALU.add)
                    nc.sync.dma_start(out=out_d[tch * 128:(tch + 1) * 128, :], in_=x2[tch][:])

    es_r1.close()
    es_x2.close()


@functools.lru_cache(maxsize=8)
def _cached_program(apply_ln_affine, add_bproj, add_b2, repeat, loop_n=0):
    return build_program(apply_ln_affine, add_bproj, add_b2, repeat, loop_n)


def _prep_shards(x, Wq, Wk, Wv, Wproj, bproj, ln1_g, ln1_b, ln2_g, ln2_b, W1, b1, W2, b2):
    wq = np.ascontiguousarray(Wq.transpose(1, 0, 2).reshape(C, C)).astype(BF)
    wk = np.ascontiguousarray(Wk.transpose(1, 0, 2).reshape(C, C)).astype(BF)
    wv = np.ascontiguousarray(Wv.transpose(1, 0, 2).reshape(C, C)).astype(BF)
    wp = Wproj.astype(BF)
    w1 = W1.astype(BF)
    w2 = W2.astype(BF)
    b1r = np.ascontiguousarray(b1.reshape(DFF // 128, 128).T).astype(np.float32)
    lnp = np.stack([ln1_g, ln1_b, ln2_g, ln2_b]).astype(np.float32)
    bpb2 = np.stack([bproj, b2]).astype(np.float32)

    in_maps = []
    for c in range(8):
        bidx, sub = c // 2, c % 2
        (lo0, lo1), (hi0, hi1) = own_ranges(sub)
        xb = x[bidx]
        x_own = np.concatenate([xb[lo0:lo1], xb[hi0:hi1]], axis=0).astype(np.float32)
        keys = np.arange(T)
        rows_b0 = np.arange(lo0, lo1)
        rows_b1 = np.arange(hi0, hi1)
        m = np.zeros((T, 512), np.float32)
        m[0:1024] = (keys[0:1024, None] <= rows_b0[None, :])
        m[1024:2048] = (keys[1024:2048, None] <= rows_b1[None, :])
        in_maps.append({
            "x_kv": np.ascontiguousarray(xb).astype(np.float32),
            "x_own": x_own,
            "maskt": m.astype(BF),
            "wq": wq, "wk": wk, "wv": wv, "wp": wp,
            "w1": w1, "w2": w2, "b1r": b1r, "lnp": lnp, "bpb2": bpb2,
        })
    return in_maps


def kernel(repeat: int = 1, loop_n: int = 0, **inputs) -> np.ndarray:
    inputs = {k: np.asarray(v) for k, v in inputs.items()}
    apply_ln_affine = not (
        np.all(inputs["ln1_g"] == 1) and np.all(inputs["ln1_b"] == 0)
        and np.all(inputs["ln2_g"] == 1) and np.all(inputs["ln2_b"] == 0))
    add_bproj = bool(np.any(inputs["bproj"] != 0))
    add_b2 = bool(np.any(inputs["b2"] != 0))
    nc = _cached_program(apply_ln_affine, add_bproj, add_b2, repeat, loop_n)
    in_maps = _prep_shards(**inputs)
    res = run_bass_kernel_spmd(nc, in_maps, list(range(8)))
    out = np.empty((B, T, C), np.float32)
    for c in range(8):
        bidx, sub = c // 2, c % 2
        (lo0, lo1), (hi0, hi1) = own_ranges(sub)
        oc = res.results[c]["out"]
        out[bidx, lo0:lo1] = oc[0:512]
        out[bidx, hi0:hi1] = oc[512:1024]
    return out


# revision 24
# speedup vs baseline: 453.9766x; 1.0677x over previous
"""Trainium2 Bass kernel for nn_Block_21406117003497 (dense transformer block).

B=4, T=2048, C=1024, H=16 heads, HS=64, DFF=4096.
8 cores: core c -> batch c//2, token-half c%2 (causally balanced row split).
Each core computes LN1+K/V for the full sequence of its batch (duplicated
within the pair -> zero inter-core communication), Q/attention/proj/MLP for
its own 1024 rows. Matmuls in bf16, accumulation in f32.
"""

import functools
from contextlib import ExitStack

import numpy as np
import ml_dtypes

import concourse.bass as bass
import concourse.mybir as mybir
import concourse.tile as tile
from concourse import bacc
from concourse.bass_utils import run_bass_kernel_spmd

F32 = mybir.dt.float32
BF16 = mybir.dt.bfloat16
AF = mybir.ActivationFunctionType
ALU = mybir.AluOpType
AX = mybir.AxisListType

B, T, C, H, HS = 4, 2048, 1024, 16, 64
DFF = 4 * C
R = 1024            # own rows per core
EPS = 1e-5
SCALE = float(C) ** -0.5
BF = ml_dtypes.bfloat16


def own_ranges(sub):
    """local row block -> absolute row ranges per sub (causally balanced)."""
    if sub == 0:
        return (0, 512), (1536, 2048)
    return (512, 1024), (1024, 1536)


def build_program(apply_ln_affine: bool, add_bproj: bool, add_b2: bool, repeat: int = 1,
                  loop_n: int = 0, variant: str = ""):
    nc = bacc.Bacc(None, target_bir_lowering=False, debug=False)

    env = {}
    env["variant"] = set(variant.split("+")) if variant else set()
    env["apply_ln_affine"] = apply_ln_affine
    env["x_kv"] = nc.dram_tensor("x_kv", [T, C], F32, kind="ExternalInput")
    env["x_own"] = nc.dram_tensor("x_own", [R, C], F32, kind="ExternalInput")
    env["maskt"] = nc.dram_tensor("maskt", [T, 512], BF16, kind="ExternalInput")
    env["wq_d"] = nc.dram_tensor("wq", [C, C], BF16, kind="ExternalInput")
    env["wk_d"] = nc.dram_tensor("wk", [C, C], BF16, kind="ExternalInput")
    env["wv_d"] = nc.dram_tensor("wv", [C, C], BF16, kind="ExternalInput")
    env["wp_d"] = nc.dram_tensor("wp", [C, C], BF16, kind="ExternalInput")
    env["w1_d"] = nc.dram_tensor("w1", [C, DFF], BF16, kind="ExternalInput")
    env["w2_d"] = nc.dram_tensor("w2", [DFF, C], BF16, kind="ExternalInput")
    env["b1r_d"] = nc.dram_tensor("b1r", [128, DFF // 128], F32, kind="ExternalInput")
    env["ln_d"] = nc.dram_tensor("lnp", [4, C], F32, kind="ExternalInput")
    env["bpb2_d"] = nc.dram_tensor("bpb2", [2, C], F32, kind="ExternalInput")
    env["out_d"] = nc.dram_tensor("out", [R, C], F32, kind="ExternalOutput")

    with tile.TileContext(nc) as tc:
        with tc.tile_pool(name="consts", bufs=1, side="left") as consts:
            env["eps_t"] = consts.tile([128, 1], F32, name="eps_t")
            nc.vector.memset(env["eps_t"][:], EPS)
            env["zeros_t"] = consts.tile([128, 512], F32, name="zeros_t")
            nc.vector.memset(env["zeros_t"][:], 0.0)
            env["b1r_t"] = consts.tile([128, DFF // 128], F32, name="b1r_t")
            nc.sync.dma_start(out=env["b1r_t"][:], in_=env["b1r_d"][:, :])
            lnb = {}
            if apply_ln_affine:
                for i, nm in enumerate(("g1", "b1", "g2", "b2")):
                    row = consts.tile([1, C], F32, name=f"lnrow_{nm}")
                    nc.sync.dma_start(out=row[:], in_=env["ln_d"][i:i + 1, :])
                    bc = consts.tile([128, C], F32, name=f"lnb_{nm}")
                    nc.gpsimd.partition_broadcast(bc[:], row[0:1, :])
                    lnb[nm] = bc
            env["lnb"] = lnb
            bias_b = {}
            for i, (nm, need) in enumerate((("bproj", add_bproj), ("b2", add_b2))):
                if need:
                    row = consts.tile([1, C], F32, name=f"brow_{nm}")
                    nc.sync.dma_start(out=row[:], in_=env["bpb2_d"][i:i + 1, :])
                    bc = consts.tile([128, C], F32, name=f"bias_{nm}")
                    nc.gpsimd.partition_broadcast(bc[:], row[0:1, :])
                    bias_b[nm] = bc
            env["bias_b"] = bias_b

            if loop_n:
                with tc.For_i(0, loop_n, 1):
                    emit_block(nc, tc, env)
            else:
                for _rep in range(repeat):
                    emit_block(nc, tc, env)
    nc.compile()
    return nc


def emit_block(nc, tc, env):
    x_kv, x_own, maskt = env["x_kv"], env["x_own"], env["maskt"]
    wq_d, wk_d, wv_d, wp_d = env["wq_d"], env["wk_d"], env["wv_d"], env["wp_d"]
    w1_d, w2_d, b1r_t = env["w1_d"], env["w2_d"], env["b1r_t"]
    out_d = env["out_d"]
    eps_t, zeros_t = env["eps_t"], env["zeros_t"]
    lnb, bias_b = env["lnb"], env["bias_b"]
    apply_ln_affine = env["apply_ln_affine"]
    V = env["variant"]

    def layernorm_rows(src_getter, n_tiles, dst_tiles, pools, gkey, bkey):
        """LN over row-major f32 [128,C] tiles -> transposed bf16 dst tiles."""
        xc_p, st_p, rm_p = pools
        for i in range(n_tiles):
            if "noln" in V:
                hrow = rm_p.tile([128, C], BF16, name="hrow", tag="hrow")
                nc.vector.memset(hrow[:, 0:C], 0.5)
                if "notrans" not in V:
                    for cj in range(8):
                        nc.sync.dma_start_transpose(
                            dst_tiles[cj][:, i * 128:(i + 1) * 128],
                            hrow[:, cj * 128:(cj + 1) * 128])
                continue
            xt = src_getter(i)
            musum = st_p.tile([128, 1], F32, name="musum", tag="musum")
            nc.vector.reduce_sum(musum[:], xt[:], axis=AX.X)
            nmu = st_p.tile([128, 1], F32, name="nmu", tag="nmu")
            nc.vector.tensor_scalar(out=nmu[:], in0=musum[:], scalar1=-1.0 / C,
                                    scalar2=None, op0=ALU.mult)
            xc = xc_p.tile([128, C], F32, name="xc", tag="xc")
            nc.scalar.activation(xc[:], xt[:], AF.Identity, bias=nmu[:, 0:1])
            sqv = xc_p.tile([128, C], F32, name="sqv", tag="sqv")
            ssq = st_p.tile([128, 1], F32, name="ssq", tag="ssq")
            nc.scalar.activation(sqv[:], xc[:], AF.Square, bias=0.0, accum_out=ssq[:])
            std = st_p.tile([128, 1], F32, name="std", tag="std")
            nc.scalar.activation(std[:], ssq[:], AF.Sqrt, bias=eps_t[:, 0:1],
                                 scale=1.0 / C)
            rstd = st_p.tile([128, 1], F32, name="rstd", tag="rstd")
            nc.vector.reciprocal(rstd[:], std[:])
            hrow = rm_p.tile([128, C], BF16, name="hrow", tag="hrow")
            if apply_ln_affine:
                tmp = xc_p.tile([128, C], F32, name="lntmp", tag="lntmp")
                nc.vector.scalar_tensor_tensor(tmp[:], xc[:], rstd[:, 0:1],
                                               lnb[gkey][:], op0=ALU.mult, op1=ALU.mult)
                nc.vector.scalar_tensor_tensor(hrow[:], tmp[:], 1.0,
                                               lnb[bkey][:], op0=ALU.mult, op1=ALU.add)
            else:
                nc.vector.tensor_scalar(out=hrow[:], in0=xc[:], scalar1=rstd[:, 0:1],
                                        scalar2=None, op0=ALU.mult)
            if "notrans" not in V:
                for cj in range(8):
                    nc.sync.dma_start_transpose(
                        dst_tiles[cj][:, i * 128:(i + 1) * 128],
                        hrow[:, cj * 128:(cj + 1) * 128])

    def load_w(pool, dram, tag):
        tiles = []
        for k in range(8):
            wt = pool.tile([128, C], BF16, name=f"{tag}{k}", tag="wsb")
            nc.sync.dma_start(out=wt[:], in_=dram[k * 128:(k + 1) * 128, :])
            tiles.append(wt)
        return tiles

    # ================= LEFT: hlnT/hoT =================
    with ExitStack() as es_hln:
        p_hlnT = es_hln.enter_context(tc.tile_pool(name="p_hlnT", bufs=8, side="left"))
        p_hoT = es_hln.enter_context(tc.tile_pool(name="p_hoT", bufs=8, side="left"))
        hlnT = [p_hlnT.tile([128, T], BF16, name=f"hlnT{i}", tag="hlnT") for i in range(8)]
        hoT = [p_hoT.tile([128, R], BF16, name=f"hoT{i}", tag="hoT") for i in range(8)]

        # ---- LN1 (x_kv -> hlnT ; x_own -> hoT), locals on right ----
        with tc.tile_pool(name="p_xin", bufs=3, side="right") as p_xin, \
             tc.tile_pool(name="p_xc", bufs=3, side="right") as p_xc, \
             tc.tile_pool(name="p_st", bufs=4, side="right") as p_st, \
             tc.tile_pool(name="p_rm", bufs=3, side="right") as p_rm:
            def load_kv(i):
                xt = p_xin.tile([128, C], F32, name="xkv_t", tag="xin")
                nc.sync.dma_start(out=xt[:], in_=x_kv[i * 128:(i + 1) * 128, :])
                return xt
            layernorm_rows(load_kv, T // 128, hlnT, (p_xc, p_st, p_rm), "g1", "b1")

            def load_own(i):
                xt = p_xin.tile([128, C], F32, name="xown_t", tag="xin")
                nc.sync.dma_start(out=xt[:], in_=x_own[i * 128:(i + 1) * 128, :])
                return xt
            layernorm_rows(load_own, R // 128, hoT, (p_xc, p_st, p_rm), "g1", "b1")

        # ================= RIGHT: QKV outputs =================
        es_qkv = ExitStack()
        p_QT = es_qkv.enter_context(tc.tile_pool(name="p_QT", bufs=8, side="right"))
        p_KT = es_qkv.enter_context(tc.tile_pool(name="p_KT", bufs=8, side="right"))
        p_V = es_qkv.enter_context(tc.tile_pool(name="p_V", bufs=16, side="right"))
        QT = [p_QT.tile([128, R], BF16, name=f"QT{i}", tag="QT") for i in range(8)]
        KT = [p_KT.tile([128, T], BF16, name=f"KT{i}", tag="KT") for i in range(8)]
        Vg = [p_V.tile([128, 16 * 65], BF16, name=f"Vg{i}", tag="Vg") for i in range(16)]
        if "noqkv" in V:
            for t_ in QT + KT + Vg:
                nc.vector.memset(t_[:, :], 0.01)

        # ---- QKV matmuls (locals nested on right) ----
        with tc.tile_pool(name="p_wqkv", bufs=9, side="right") as p_w, \
             tc.tile_pool(name="ps_qkv", bufs=4, space="PSUM") as ps_a:
            for dram, dstT, rhsT, n_tok in (() if "noqkv" in V else
                                            ((wq_d, QT, hoT, R), (wk_d, KT, hlnT, T))):
                wt = load_w(p_w, dram, "wqk")
                for m in range(8):
                    for nbp in range(n_tok // 1024):
                        psA = ps_a.tile([128, 512], F32, name="qk_psA", tag="ps_a")
                        psB = ps_a.tile([128, 512], F32, name="qk_psB", tag="ps_a")
                        for k in range(8):
                            lhs = wt[k][:, m * 128:(m + 1) * 128]
                            nc.tensor.matmul(psA[:], lhs,
                                             rhsT[k][:, (2 * nbp) * 512:(2 * nbp + 1) * 512],
                                             start=(k == 0), stop=(k == 7))
                            nc.tensor.matmul(psB[:], lhs,
                                             rhsT[k][:, (2 * nbp + 1) * 512:(2 * nbp + 2) * 512],
                                             start=(k == 0), stop=(k == 7))
                        nc.vector.tensor_copy(dstT[m][:, (2 * nbp) * 512:(2 * nbp + 1) * 512], psA[:])
                        nc.vector.tensor_copy(dstT[m][:, (2 * nbp + 1) * 512:(2 * nbp + 2) * 512], psB[:])

            wt = load_w(p_w, wv_d, "wv")
            for tch in (() if "noqkv" in V else range(16)):
                nc.gpsimd.memset(
                    Vg[tch][:, 0:16 * 65].rearrange("p (h d) -> p h d", d=65)[:, :, 64:65], 1.0)
                psA = ps_a.tile([128, 512], F32, name="v_psA", tag="ps_a")
                psB = ps_a.tile([128, 512], F32, name="v_psB", tag="ps_a")
                for k in range(8):
                    lhs = hlnT[k][:, tch * 128:(tch + 1) * 128]
                    nc.tensor.matmul(psA[:], lhs, wt[k][:, 0:512], start=(k == 0), stop=(k == 7))
                    nc.tensor.matmul(psB[:], lhs, wt[k][:, 512:1024], start=(k == 0), stop=(k == 7))
                for j, ps in ((0, psA), (1, psB)):
                    dst = Vg[tch][:, j * 8 * 65:(j + 1) * 8 * 65].rearrange(
                        "p (h d) -> p h d", d=65)[:, :, 0:64]
                    nc.vector.tensor_copy(dst, ps[:].rearrange("p (h d) -> p h d", d=64))
    # es_hln closed here (hlnT/hoT freed, left side)

    # ================= LEFT: oT =================
    with ExitStack() as es_oT:
        p_oT = es_oT.enter_context(tc.tile_pool(name="p_oT", bufs=8, side="left"))
        oT = [p_oT.tile([128, R], BF16, name=f"oT{i}", tag="oT") for i in range(8)]
        if "noattn" in V:
            for t_ in oT:
                nc.vector.memset(t_[:, :], 0.01)

        # ---- attention (locals nested on right, inside QKV-outs) ----
        with tc.tile_pool(name="p_mask", bufs=16, side="right") as p_mask, \
             tc.tile_pool(name="p_E", bufs=36, side="right") as p_E, \
             tc.tile_pool(name="p_inv", bufs=4, side="right") as p_inv, \
             tc.tile_pool(name="ps_s", bufs=4, space="PSUM") as ps_s, \
             tc.tile_pool(name="ps_av", bufs=4, space="PSUM") as ps_av:
            mk = []
            for kt in range(16):
                m = p_mask.tile([128, 512], BF16, name=f"mk{kt}", tag="mk")
                nc.sync.dma_start(out=m[:], in_=maskt[kt * 128:(kt + 1) * 128, :])
                mk.append(m)

            for hp in (() if "noattn" in V else range(8)):
                for b in (0, 1):
                    n_kt = 8 if b == 0 else 16
                    # phase 1: all score matmuls -> exp -> mask; E tiles stay in SBUF
                    Es = []
                    for kt in range(n_kt):
                        masked = (b == 0) or (kt >= 8)
                        for hh in (0, 1):
                            kslc = KT[hp][hh * 64:(hh + 1) * 64, kt * 128:(kt + 1) * 128]
                            qslc = QT[hp][hh * 64:(hh + 1) * 64, b * 512:(b + 1) * 512]
                            sps = ps_s.tile([128, 512], F32, name="s_ps", tag="ps_s")
                            nc.tensor.matmul(sps[:], kslc, qslc, start=True, stop=True)
                            E = p_E.tile([128, 512], BF16, name="E", tag="E")
                            nc.scalar.activation(E[:], sps[:], AF.Exp, scale=SCALE)
                            if masked and "nomask" not in V:
                                nc.vector.tensor_mul(E[:], E[:], mk[kt][:])
                            Es.append(E)
                    # phase 2: all av matmuls (accumulate over kt)
                    oa = ps_av.tile([128, 512], F32, name="av_psA", tag="ps_av")
                    ob = ps_av.tile([128, 512], F32, name="av_psB", tag="ps_av")
                    for kt in range(n_kt):
                        for hh, ops in ((0, oa), (1, ob)):
                            h = 2 * hp + hh
                            nc.tensor.matmul(ops[0:65, :], Vg[kt][:, h * 65:h * 65 + 65],
                                             Es[2 * kt + hh][:],
                                             start=(kt == 0), stop=(kt == n_kt - 1))
                    for hh, ops in ((0, oa), (1, ob)):
                        invd = p_inv.tile([1, 512], F32, name="invd", tag="invd")
                        nc.vector.reciprocal(invd[:], ops[64:65, :])
                        invb = p_inv.tile([64, 512], F32, name="invb", tag="invb")
                        nc.gpsimd.partition_broadcast(invb[:], invd[0:1, :])
                        nc.vector.tensor_mul(
                            oT[hp][hh * 64:(hh + 1) * 64, b * 512:(b + 1) * 512],
                            ops[0:64, :], invb[:])

        es_qkv.close()  # QT/KT/V freed (right side)

        # ================= RIGHT: x2 =================
        es_x2 = ExitStack()
        p_x2 = es_x2.enter_context(tc.tile_pool(name="p_x2", bufs=8, side="right"))
        x2 = [p_x2.tile([128, C], F32, name=f"x2_{i}", tag="x2") for i in range(8)]
        if "noproj" in V:
            for t_ in x2:
                nc.vector.memset(t_[:, :], 0.01)

        # ---- proj + residual (locals nested on right, inside x2) ----
        with tc.tile_pool(name="p_wp", bufs=9, side="right") as p_wp, \
             tc.tile_pool(name="p_xo", bufs=3, side="right") as p_xo, \
             tc.tile_pool(name="ps_pj", bufs=4, space="PSUM") as ps_pj:
            wt = load_w(p_wp, wp_d, "wp")
            for tch in (() if "noproj" in V else range(8)):
                xo = p_xo.tile([128, C], F32, name="xo", tag="xo")
                nc.sync.dma_start(out=xo[:], in_=x_own[tch * 128:(tch + 1) * 128, :])
                psA = ps_pj.tile([128, 512], F32, name="pj_psA", tag="ps_pj")
                psB = ps_pj.tile([128, 512], F32, name="pj_psB", tag="ps_pj")
                for k in range(8):
                    lhs = oT[k][:, tch * 128:(tch + 1) * 128]
                    nc.tensor.matmul(psA[:], lhs, wt[k][:, 0:512], start=(k == 0), stop=(k == 7))
                    nc.tensor.matmul(psB[:], lhs, wt[k][:, 512:1024], start=(k == 0), stop=(k == 7))
                for j, ps in ((0, psA), (1, psB)):
                    sl = slice(j * 512, (j + 1) * 512)
                    nc.vector.scalar_tensor_tensor(x2[tch][:, sl], ps[:], 1.0, xo[:, sl],
                                                   op0=ALU.mult, op1=ALU.add)
                    if "bproj" in bias_b:
                        nc.vector.scalar_tensor_tensor(x2[tch][:, sl], x2[tch][:, sl], 1.0,
                                                       bias_b["bproj"][:, sl],
                                                       op0=ALU.mult, op1=ALU.add)
    # es_oT closed (left)

    # ================= LEFT: h2T =================
    with ExitStack() as es_h2:
        p_h2T = es_h2.enter_context(tc.tile_pool(name="p_h2T", bufs=8, side="left"))
        h2T = [p_h2T.tile([128, R], BF16, name=f"h2T{i}", tag="h2T") for i in range(8)]
        with tc.tile_pool(name="p_xc2", bufs=3, side="left") as p_xc2, \
             tc.tile_pool(name="p_st2", bufs=4, side="left") as p_st2, \
             tc.tile_pool(name="p_rm2", bufs=3, side="left") as p_rm2:
            layernorm_rows(lambda i: x2[i], 8, h2T, (p_xc2, p_st2, p_rm2), "g2", "b2")

        # ================= RIGHT: relu1T (inside x2) =================
        es_r1 = ExitStack()
        p_r1 = es_r1.enter_context(tc.tile_pool(name="p_r1", bufs=32, side="right"))
        r1T = [p_r1.tile([128, R], BF16, name=f"r1T{i}", tag="r1T") for i in range(32)]
        ps_m = es_r1.enter_context(tc.tile_pool(name="ps_m", bufs=4, space="PSUM"))

        with tc.tile_pool(name="p_w1", bufs=16, side="right") as p_w1:
            for dblock in (() if "nomlp" in V else range(8)):
                w1c = []
                for k in range(8):
                    wt1 = p_w1.tile([128, 512], BF16, name=f"w1c{dblock}_{k}", tag="w1c")
                    nc.sync.dma_start(out=wt1[:], in_=w1_d[k * 128:(k + 1) * 128,
                                                           dblock * 512:(dblock + 1) * 512])
                    w1c.append(wt1)
                for dc in range(4):
                    g = dblock * 4 + dc
                    psA = ps_m.tile([128, 512], F32, name="m1_psA", tag="ps_m")
                    psB = ps_m.tile([128, 512], F32, name="m1_psB", tag="ps_m")
                    for k in range(8):
                        lhs = w1c[k][:, dc * 128:(dc + 1) * 128]
                        nc.tensor.matmul(psA[:], lhs, h2T[k][:, 0:512],
                                         start=(k == 0), stop=(k == 7))
                        nc.tensor.matmul(psB[:], lhs, h2T[k][:, 512:1024],
                                         start=(k == 0), stop=(k == 7))
                    for j, ps in ((0, psA), (1, psB)):
                        nc.vector.scalar_tensor_tensor(
                            r1T[g][:, j * 512:(j + 1) * 512], ps[:], b1r_t[:, g:g + 1],
                            zeros_t[:], op0=ALU.add, op1=ALU.max)
    # es_h2 closed (left)

    with tc.tile_pool(name="p_w2", bufs=18, side="right") as p_w2:
        for kh in range(2) if "nomlp" not in V else ():
            w2c = []
            for k in range(16):
                kk = kh * 16 + k
                wt2 = p_w2.tile([128, C], BF16, name=f"w2c{kh}_{k}", tag="w2c")
                nc.sync.dma_start(out=wt2[:], in_=w2_d[kk * 128:(kk + 1) * 128, :])
                w2c.append(wt2)
            for tch in range(8):
                psA = ps_m.tile([128, 512], F32, name="m2_psA", tag="ps_m")
                psB = ps_m.tile([128, 512], F32, name="m2_psB", tag="ps_m")
                for k in range(16):
                    kk = kh * 16 + k
                    lhs = r1T[kk][:, tch * 128:(tch + 1) * 128]
                    nc.tensor.matmul(psA[:], lhs, w2c[k][:, 0:512],
                                     start=(k == 0), stop=(k == 15))
                    nc.tensor.matmul(psB[:], lhs, w2c[k][:, 512:1024],
                                     start=(k == 0), stop=(k == 15))
                for j, ps in ((0, psA), (1, psB)):
                    sl = slice(j * 512, (j + 1) * 512)
                    nc.vector.scalar_tensor_tensor(x2[tch][:, sl], ps[:], 1.0,
                                                   x2[tch][:, sl], op0=ALU.mult, op1=ALU.add)
                if kh == 1:
                    if "b2" in bias_b:
                        for j in range(2):
                            sl = slice(j * 512, (j + 1) * 512)
                            nc.vector.scalar_tensor_tensor(
                                x2[tch][:, sl], x2[tch][:, sl], 1.0,
                                bias_b["b2"][:, sl], op0=ALU.mult, op1=ALU.add)
                    nc.sync.dma_start(out=out_d[tch * 128:(tch + 1) * 128, :], in_=x2[tch][:])

    es_r1.close()
    es_x2.close()


@functools.lru_cache(maxsize=16)
def _cached_program(apply_ln_affine, add_bproj, add_b2, repeat, loop_n=0, variant=""):
    return build_program(apply_ln_affine, add_bproj, add_b2, repeat, loop_n, variant)


def _prep_shards(x, Wq, Wk, Wv, Wproj, bproj, ln1_g, ln1_b, ln2_g, ln2_b, W1, b1, W2, b2):
    wq = np.ascontiguousarray(Wq.transpose(1, 0, 2).reshape(C, C)).astype(BF)
    wk = np.ascontiguousarray(Wk.transpose(1, 0, 2).reshape(C, C)).astype(BF)
    wv = np.ascontiguousarray(Wv.transpose(1, 0, 2).reshape(C, C)).astype(BF)
    wp = Wproj.astype(BF)
    w1 = W1.astype(BF)
    w2 = W2.astype(BF)
    b1r = np.ascontiguousarray(b1.reshape(DFF // 128, 128).T).astype(np.float32)
    lnp = np.stack([ln1_g, ln1_b, ln2_g, ln2_b]).astype(np.float32)
    bpb2 = np.stack([bproj, b2]).astype(np.float32)

    in_maps = []
    for c in range(8):
        bidx, sub = c // 2, c % 2
        (lo0, lo1), (hi0, hi1) = own_ranges(sub)
        xb = x[bidx]
        x_own = np.concatenate([xb[lo0:lo1], xb[hi0:hi1]], axis=0).astype(np.float32)
        keys = np.arange(T)
        rows_b0 = np.arange(lo0, lo1)
        rows_b1 = np.arange(hi0, hi1)
        m = np.zeros((T, 512), np.float32)
        m[0:1024] = (keys[0:1024, None] <= rows_b0[None, :])
        m[1024:2048] = (keys[1024:2048, None] <= rows_b1[None, :])
        in_maps.append({
            "x_kv": np.ascontiguousarray(xb).astype(np.float32),
            "x_own": x_own,
            "maskt": m.astype(BF),
            "wq": wq, "wk": wk, "wv": wv, "wp": wp,
            "w1": w1, "w2": w2, "b1r": b1r, "lnp": lnp, "bpb2": bpb2,
        })
    return in_maps


def kernel(repeat: int = 1, loop_n: int = 0, variant: str = "", **inputs) -> np.ndarray:
    inputs = {k: np.asarray(v) for k, v in inputs.items()}
    apply_ln_affine = not (
        np.all(inputs["ln1_g"] == 1) and np.all(inputs["ln1_b"] == 0)
        and np.all(inputs["ln2_g"] == 1) and np.all(inputs["ln2_b"] == 0))
    add_bproj = bool(np.any(inputs["bproj"] != 0))
    add_b2 = bool(np.any(inputs["b2"] != 0))
    nc = _cached_program(apply_ln_affine, add_bproj, add_b2, repeat, loop_n, variant)
    in_maps = _prep_shards(**inputs)
    res = run_bass_kernel_spmd(nc, in_maps, list(range(8)))
    out = np.empty((B, T, C), np.float32)
    for c in range(8):
        bidx, sub = c // 2, c % 2
        (lo0, lo1), (hi0, hi1) = own_ranges(sub)
        oc = res.results[c]["out"]
        out[bidx, lo0:lo1] = oc[0:512]
        out[bidx, hi0:hi1] = oc[512:1024]
    return out
